# revision 1
# baseline (speedup 1.0000x reference)
"""Adaptive embedding lookup (4 vocab buckets, per-bucket projection) on 8 TRN2 cores.

Strategy: token-parallel. Tokens are bucketed by vocab range on host; each
bucket's token list is padded to a multiple of 8 and dealt round-robin so all
8 cores get identical static shapes (SPMD). On device, each core:
  - indirect-DMA-gathers its tokens' embedding rows from the full tables in DRAM
  - transposes gathered tiles on the TensorEngine (tokens->partitions becomes
    embed-dim->partitions, as matmul contracts over partitions)
  - matmuls against the host-pre-transposed, EMB_SCALE-folded projections (bf16)
  - writes its [rows, 1024] output shard contiguously (bucket-major)
Host then inverse-permutes the 8 shards into the full [B, S, 1024] output.
"""
import sys
import types

import numpy as np

if "/opt/trn_rl_repo" not in sys.path:
    sys.path.insert(0, "/opt/trn_rl_repo")

import ml_dtypes  # noqa: E402
from concourse import bacc, bass, mybir, tile  # noqa: E402
from concourse.bass_utils import run_bass_kernel_spmd  # noqa: E402
from concourse.masks import make_identity  # noqa: E402

N_CORES = 8
P = 128
CUTS = [0, 20000, 40000, 200000, 267735]
N_BUCKETS = 4
D_PROJ = 1024
EMB_SCALE = float(D_PROJ) ** 0.5

F32 = mybir.dt.float32
BF16 = mybir.dt.bfloat16
I32 = mybir.dt.int32


def _cdiv(a, b):
    return -(-a // b)


def _build_graph(m, d_emb, v_emb, T):
    """Build the per-core SPMD graph.

    m: per-core token count per bucket (same on all cores)
    d_emb: embed dim per bucket; v_emb: vocab rows per bucket
    T: total number of 128-token gather tiles
    """
    R = sum(m)  # output rows per core
    nc = bacc.Bacc(None, target_bir_lowering=False, debug=False)

    idx_p = nc.declare_dram_parameter("idx", [P, T], I32, isOutput=False)
    emb_p = [
        nc.declare_dram_parameter(f"emb{b}", [v_emb[b], d_emb[b]], F32, isOutput=False)
        for b in range(N_BUCKETS)
    ]
    # projT params: host lays them out so a straight DMA gives the rhs chunks.
    pt_p = []
    for b in range(N_BUCKETS):
        kc = _cdiv(d_emb[b], P)
        if d_emb[b] >= P:
            shape = [P, kc * D_PROJ]
        else:
            shape = [d_emb[b], D_PROJ]
        pt_p.append(nc.declare_dram_parameter(f"pt{b}", shape, BF16, isOutput=False))
    out_p = nc.declare_dram_parameter("out", [R, D_PROJ], F32, isOutput=True)

    with tile.TileContext(nc) as tc:
        with (
            tc.tile_pool(name="persist", bufs=1) as pp,
            tc.tile_pool(name="gather", bufs=2) as gp,
            tc.tile_pool(name="lhsT", bufs=2) as lp,
            tc.tile_pool(name="outs", bufs=3) as op,
            tc.tile_pool(name="ps_tr", bufs=2, space="PSUM") as ps_tr,
            tc.tile_pool(name="ps_mm", bufs=4, space="PSUM") as ps_mm,
        ):
            ident = pp.tile([P, P], F32)
            make_identity(nc, ident[:])

            idx_sb = pp.tile([P, T], I32)
            nc.sync.dma_start(out=idx_sb[:], in_=idx_p[:])

            pt_sb = []
            for b in range(N_BUCKETS):
                d = d_emb[b]
                kc = _cdiv(d, P)
                if d >= P:
                    t_ = pp.tile([P, kc * D_PROJ], BF16, tag=f"pt{b}")
                    nc.scalar.dma_start(out=t_[:], in_=pt_p[b][:])
                else:
                    t_ = pp.tile([P, D_PROJ], BF16, tag=f"pt{b}")
                    nc.scalar.dma_start(out=t_[:d, :], in_=pt_p[b][:])
                pt_sb.append(t_)

            t = 0
            row_start = 0
            for b in range(N_BUCKETS):
                d = d_emb[b]
                kc = _cdiv(d, P)
                n_tiles = _cdiv(m[b], P)
                for j in range(n_tiles):
                    rows = min(P, m[b] - j * P)
                    g = gp.tile([P, d], F32, tag=f"g{b}")
                    nc.gpsimd.indirect_dma_start(
                        out=g[:],
                        out_offset=None,
                        in_=emb_p[b][:],
                        in_offset=bass.IndirectOffsetOnAxis(
                            ap=idx_sb[:, t : t + 1], axis=0
                        ),
                    )
                    lhsT = lp.tile([P, kc * P], BF16, tag=f"l{b}")
                    for k in range(kc):
                        cw = min(P, d - k * P)
                        trp = ps_tr.tile([P, P], F32, tag="tr")
                        nc.tensor.transpose(
                            out=trp[:cw, :P],
                            in_=g[:, k * P : k * P + cw],
                            identity=ident[:],
                        )
                        nc.vector.tensor_copy(
                            out=lhsT[:cw, k * P : (k + 1) * P], in_=trp[:cw, :P]
                        )
                    out_sb = op.tile([P, D_PROJ], F32, tag="o")
                    for h in range(D_PROJ // 512):
                        mm = ps_mm.tile([P, 512], F32, tag="mm")
                        for k in range(kc):
                            cw = min(P, d - k * P)
                            if d >= P:
                                rhs = pt_sb[b][
                                    :, k * D_PROJ + h * 512 : k * D_PROJ + (h + 1) * 512
                                ]
                            else:
                                rhs = pt_sb[b][:cw, h * 512 : (h + 1) * 512]
                            nc.tensor.matmul(
                                mm[:, :],
                                lhsT[:cw, k * P : (k + 1) * P],
                                rhs,
                                start=(k == 0),
                                stop=(k == kc - 1),
                            )
                        if h == 0:
                            nc.vector.tensor_copy(
                                out=out_sb[:, h * 512 : (h + 1) * 512], in_=mm[:, :]
                            )
                        else:
                            nc.scalar.activation(
                                out=out_sb[:, h * 512 : (h + 1) * 512],
                                in_=mm[:, :],
                                func=mybir.ActivationFunctionType.Copy,
                            )
                    nc.sync.dma_start(
                        out=out_p[row_start : row_start + rows, :],
                        in_=out_sb[:rows, :],
                    )
                    row_start += rows
                    t += 1

    nc.compile()
    return nc


def kernel(inp, emb0, emb1, emb2, emb3, proj0, proj1, proj2, proj3):
    embs = [np.ascontiguousarray(e, dtype=np.float32) for e in (emb0, emb1, emb2, emb3)]
    projs = [proj0, proj1, proj2, proj3]
    d_emb = [e.shape[1] for e in embs]
    v_emb = [e.shape[0] for e in embs]

    inp = np.asarray(inp)
    orig_shape = inp.shape
    flat = inp.reshape(-1).astype(np.int64)
    N = flat.shape[0]

    bucket = np.digitize(flat, CUTS[1:-1])  # 0..3
    pos_pad = []   # padded global positions per bucket (-1 = pad slot)
    loc_pad = []   # padded local (within-table) indices per bucket
    m = []
    for b in range(N_BUCKETS):
        pos = np.nonzero(bucket == b)[0]
        loc = (flat[pos] - CUTS[b]).astype(np.int64)
        loc = np.clip(loc, 0, v_emb[b] - 1)
        n = len(pos)
        npad = _cdiv(n, N_CORES) * N_CORES
        pos_full = np.full(npad, -1, dtype=np.int64)
        loc_full = np.zeros(npad, dtype=np.int64)
        pos_full[:n] = pos
        loc_full[:n] = loc
        pos_pad.append(pos_full)
        loc_pad.append(loc_full)
        m.append(npad // N_CORES)

    n_tiles = [_cdiv(mb, P) for mb in m]
    T = sum(n_tiles)

    # per-core index arrays [P, T]: column t holds the 128 table-row indices
    # for gather tile t (bucket-major, zero-padded past each bucket's count)
    idx_arrs = []
    for c in range(N_CORES):
        cols = np.zeros((T, P), dtype=np.int32)
        t = 0
        for b in range(N_BUCKETS):
            locs_c = loc_pad[b][c::N_CORES]  # [m[b]]
            padded = np.zeros(n_tiles[b] * P, dtype=np.int32)
            padded[: m[b]] = locs_c.astype(np.int32)
            cols[t : t + n_tiles[b]] = padded.reshape(n_tiles[b], P)
            t += n_tiles[b]
        idx_arrs.append(np.ascontiguousarray(cols.T))  # [P, T]

    # projT host prep: transpose, fold EMB_SCALE, bf16, chunk-major layout
    pts = []
    for b in range(N_BUCKETS):
        d = d_emb[b]
        ptb = (np.asarray(projs[b], dtype=np.float32).T * EMB_SCALE)  # [d, D_PROJ]
        if d >= P:
            kc = d // P
            ptb = ptb.reshape(kc, P, D_PROJ).transpose(1, 0, 2).reshape(P, kc * D_PROJ)
        pts.append(np.ascontiguousarray(ptb.astype(ml_dtypes.bfloat16)))

    nc = _build_graph(m, d_emb, v_emb, T)

    in_maps = []
    for c in range(N_CORES):
        im = {"idx": idx_arrs[c]}
        for b in range(N_BUCKETS):
            im[f"emb{b}"] = embs[b]
            im[f"pt{b}"] = pts[b]
        in_maps.append(im)

    res = run_bass_kernel_spmd(nc, in_maps, core_ids=list(range(N_CORES)))

    out_full = np.zeros((N, D_PROJ), dtype=np.float32)
    for c in range(N_CORES):
        shard = res.results[c]["out"]  # [R, D_PROJ]
        row = 0
        for b in range(N_BUCKETS):
            pos_c = pos_pad[b][c::N_CORES]  # [m[b]]
            valid = pos_c >= 0
            out_full[pos_c[valid]] = shard[row : row + m[b]][valid]
            row += m[b]
    return out_full.reshape(*orig_shape, D_PROJ)
